# revision 35
# baseline (speedup 1.0000x reference)
"""BertAttention Trainium2 kernel v3 (8 NeuronCores, SPMD, no collectives).

Sharding: DP over batch (2) x sequence-parallel over 512-row query blocks (4).
All heavy matmuls run fp8e4 DoubleRow (2 contraction subtiles per instruction,
0.5 cyc/output-column). Changes vs v2 (166.4us -> 156.9us):

  - attention mask folded into the softmax exp as a per-partition bias
    (Act: exp(scale*s + mask); DVE Schraudolph: u8 = s*A + (56+8*log2e*mask)
    bitcast fp8e4), so the V evac is a pure fp8 cast and the VA denominator
    column is a 1.0 memset.
  - q and k projections for a pair share one qk_sb tile [128, 2560]; the
    DoubleRow rearrange is 4 merged DMAs per pair (one per (head, subtile))
    instead of 20 - HWDGE costs ~620ns per DMA op and fully serializes, so
    DMA count dominated the old schedule (190 -> ~60 DMAs).
  - leading score tiles per pair (NODR) run non-DoubleRow straight from
    qk_sb (64-partition contraction, 2x PE cost) so pair 0 needs no
    rearrange at all and later pairs tolerate qkdr DMA latency.
  - GPSIMD cannot access PSUM, so every PSUM-draining op (exps, evacs,
    norm) is balanced across Act/DVE via the per-pair tables above; Pool
    only gets SBUF-side work (VA init, LayerNorm tail pieces). Per-slot
    emission is readiness-ordered: independent V/proj matmuls first, the
    previous slot's evacs, then the score->exp chain, staged prev-pair
    normalization, deferred ctx accumulation last.
  - ctx normalization: the broadcast ones-vector holds 32.0 (folds the fp8
    scale); rb copy on Act (one PSUM input per op limit), multiply on DVE.
  - tail: out-proj evac + residual fused into one stt whose accum_out also
    yields sum(h) for the LayerNorm mean; variance alternates between Act
    Square+accum and DVE bn_stats per output tile; with gamma==1/beta==0
    (always true for this dataset, checked at runtime) the normalize step
    is a single two-scalar tensor_scalar per column half, DVE + Pool, with
    per-half output DMAs.

The steady state is limited by two near-equal walls: Act+DVE elementwise
capacity (~230us of PSUM-drain work over 2 engines) and the s2
double-buffer exp chain (mm 214 + drain 183 + exp ~1100 + ack 250 =
~1.7us per two score tiles; only 2 chains fit in PSUM next to the ctx
accumulators and the proj/V scratch).
"""

import numpy as np
import ml_dtypes

import bass_rust as _br
import concourse.bass as bass
import concourse.tile as tile
from concourse import mybir
from concourse.bass_utils import run_bass_kernel_spmd

F32 = mybir.dt.float32
F32R = mybir.dt.float32r
F8 = mybir.dt.float8e4
U8 = mybir.dt.uint8
DR = mybir.MatmulPerfMode.DoubleRow
ADD = mybir.AluOpType.add
MULT = mybir.AluOpType.mult
SUB = mybir.AluOpType.subtract

S = 2048
H = 1024
P = 128
SQ = 512          # query rows per core
NKT = S // P      # 16 sk tiles
HC = H // P       # 8 h-chunks
NPAIR = 8         # head pairs
WS = 32.0         # fp8 weight scale
EPS = 1e-12
EXP_SCALE = 0.125 / (WS * WS)     # 2^-13
OUT_SCALE = 1.0 / (WS * WS * WS)  # 2^-15
SCH_A = 8 * 1.4426950408889634 * EXP_SCALE
SCH_B = 56.0

# ---- engine assignment tables ('a' = Act, 'd' = DVE) ----
# GPSIMD cannot touch PSUM, so every PSUM-draining op lives on Act or DVE;
# Pool only gets SBUF->SBUF work (VA init, LayerNorm tail pieces).
# softmax exp engine per (pair, sk-tile): Act on these t, DVE otherwise.
EXP_A = [
    {0, 2, 4, 6, 8, 10, 12, 14},               # pair 0
    {0, 2, 4, 6, 8, 10, 12, 14, 15},           # pair 1
    {0, 2, 4, 6, 8, 10, 12, 14, 15},           # pair 2
    {0, 2, 4, 6, 8, 10, 12, 14, 15},           # pair 3
    {0, 2, 4, 6, 8, 10, 12, 14, 15},           # pair 4
    {0, 2, 4, 6, 8, 10, 12, 14, 15},           # pair 5
    {0, 2, 4, 6, 8, 10, 12, 14, 15},           # pair 6
    {0, 2, 4, 6, 8, 10, 12, 14, 15},           # pair 7
]
# V evac engine per sk-tile (group 0 runs in pair 0, group 1 in pairs 1-3)
V_ENG = {0: ["a" if t in (1, 3, 5, 7, 9, 11, 13) else "d" for t in range(NKT)],
         1: ["a" if t % 4 != 1 else "d" for t in range(NKT)]}
# q / k-block evac engines per pair (proj for pair m+1 emitted during pair m)
Q_ENG = ["a"] * 8
K_ENG = [["d", "a", "d", "a"]] * 8
# pair 0's own late k blocks (emitted in its first slots)
K0_ENG = ["a", "a", "d"]
# V-projection emission schedule: group 0 just-in-time inside pair 0;
# group 1 spread over pairs 1-3 (list of t per (pair, t2 slot)).
V_SCHED = {
    1: {0: (0, 1), 1: (2, 3)},
    2: {0: (4, 5), 1: (6, 7)},
    3: {0: (8, 9), 1: (10, 11)},
    4: {0: (12, 13), 1: (14, 15)},
}
# number of leading score tiles per pair that run non-DoubleRow from qk_sb
# (pair 0 fully, pair 1 until its qkdr DMA - issued at its slot 0 - lands)
NODR = (16, 6, 2, 0, 0, 0, 0, 0)
# ctx emission deferral in slots (WAR vs prev pair's norm reads of the
# recycled PSUM accumulators; also relaxes the V-evac deadline)
CTX_DEFER = 5
# output tiles whose LayerNorm variance runs as Act Square+accum (others bn_stats on DVE)
VAR_SQ_STS = (0, 2)
# engine for the second ln output half ('d' or 'p')
OB1_ENG = "p"

_wait_ctr = [0]


def _split_excess_waits(nc, limit=1):
    """walrus in this container rejects >1-2 sem waits on several opcode
    structs; move excess waits onto same-engine NoOps inserted just before."""
    for f in nc.m.functions:
        for bb in f.blocks:
            insts = bb.instructions
            out = []
            dirty = False
            for inst in insts:
                si = inst.sync_info
                waits = list(si.on_wait) if si and si.on_wait else []
                if len(waits) > limit and inst.engine != mybir.EngineType.Unassigned:
                    for i in range(0, len(waits) - limit, limit):
                        _wait_ctr[0] += 1
                        nop = _br.InstNoOp(
                            name=f"I-waitsplit-{_wait_ctr[0]}", ins=[], outs=[]
                        )
                        nop.engine = inst.engine
                        nop.sync_info = mybir.SyncInfo(
                            on_wait=waits[i : i + limit], on_update=[]
                        )
                        out.append(nop)
                    si.on_wait = waits[len(waits) - limit :]
                    dirty = True
                out.append(inst)
            if dirty:
                bb.instructions = out
    return nc


def _ap(t, off, dims):
    """Strided AP over a tile's partition range: dims = [[stride, count], ...]."""
    return bass.AP(
        tensor=t.tensor,
        offset=t.offset + off,
        ap=[list(t.ap[0])] + [list(d) for d in dims],
    )


# vaug layout per partition: [t2(8), j(2), m(8), h(2), c(96)]
VA_C = 96
VA_H = VA_C            # stride of h
VA_M = 2 * VA_C        # 192
VA_J = 8 * VA_M        # 1536
VA_T2 = 2 * VA_J       # 3072
VA_TOTAL = 8 * VA_T2   # 24576

# qk_sb layout: [128, 2560]: q cols 0:512, k block n cols 512+512n
QKW = 2560
# qkdr layout: [32, h(2) x j(2) x 2560]
QKDR_J = QKW
QKDR_H = 2 * QKW


DEBUG = False


def build_nc(ln_trivial=True):
    nc = bass.Bass()

    xT_d = nc.dram_tensor("xT", [P, HC, S], F8, kind="ExternalInput")
    xq_d = nc.dram_tensor("xq", [SQ, H], F32, kind="ExternalInput")  # +bo+bv@Wo
    wq_d = nc.dram_tensor("wq", [NPAIR, P, HC * P], F8, kind="ExternalInput")
    wk_d = nc.dram_tensor("wk", [NPAIR, P, HC * P], F8, kind="ExternalInput")
    wv_d = nc.dram_tensor("wv", [2, P, HC * 512], F8, kind="ExternalInput")
    wo_d = nc.dram_tensor("wo", [P, HC * H], F8, kind="ExternalInput")
    bqk_d = nc.dram_tensor("bqk", [P, 16], F32, kind="ExternalInput")  # 32*(bq|bk)
    gamma_d = nc.dram_tensor("gamma", [H], F32, kind="ExternalInput")
    beta_d = nc.dram_tensor("beta", [H], F32, kind="ExternalInput")
    # raw attention mask per sk tile, and 56 + 8*log2e*mask (Schraudolph bias)
    mk_d = nc.dram_tensor("mk", [P, NKT], F32, kind="ExternalInput")
    mks_d = nc.dram_tensor("mks", [P, NKT], F32, kind="ExternalInput")
    out_d = nc.dram_tensor("out", [SQ, H], F32, kind="ExternalOutput")
    if DEBUG:
        dva_d = nc.dram_tensor("dva", [P, VA_TOTAL], F8, kind="ExternalOutput")
        dctxT_d = nc.dram_tensor("dctxT", [P, NPAIR * SQ], F8, kind="ExternalOutput")
        dqk_d = nc.dram_tensor("dqk", [P, QKW], F8, kind="ExternalOutput")
        dqkdr_d = nc.dram_tensor("dqkdr", [32, 2 * QKDR_H], F8, kind="ExternalOutput")
        dpt_d = nc.dram_tensor("dpt", [P, 2048], F8, kind="ExternalOutput")
        dh_d = nc.dram_tensor("dh", [P, H], F32, kind="ExternalOutput")

    with tile.TileContext(nc) as tc, nc.allow_low_precision(
        reason="fp8 DoubleRow matmuls; accumulation stays fp32 in PSUM"
    ):
        consts = tc.alloc_tile_pool(name="consts", bufs=1)
        xT_pool = tc.alloc_tile_pool(name="xT", bufs=1)
        va_pool = tc.alloc_tile_pool(name="va", bufs=1)
        wv_pool = tc.alloc_tile_pool(name="wv", bufs=1)
        ctxT_pool = tc.alloc_tile_pool(name="ctxT", bufs=1)
        xq_pool = tc.alloc_tile_pool(name="xq", bufs=1)
        w_pool = tc.alloc_tile_pool(name="w", bufs=3)
        kv_pool = tc.alloc_tile_pool(name="kv", bufs=2)
        pt_pool = tc.alloc_tile_pool(name="pt", bufs=6)
        r_pool = tc.alloc_tile_pool(name="r", bufs=4)
        ln_pool = tc.alloc_tile_pool(name="ln", bufs=2)
        ps_mm = tc.alloc_tile_pool(name="ps_mm", bufs=2, space="PSUM")
        ps_s = tc.alloc_tile_pool(name="ps_s", bufs=2, space="PSUM")
        ps_ctx = tc.alloc_tile_pool(name="ps_ctx", bufs=2, space="PSUM")

        # ---- critical-path DMAs first (HWDGE is ~620ns PER DMA op and the
        # transfers fully serialize on the DMA engines, so order = priority):
        # xT query cols, then wq0/wk0, then consts ----
        xT_all = xT_pool.tile([P, HC * S], F8, name="xT_all", tag="xT_all")
        xT_v = xT_all.rearrange("p (c s) -> p c s", c=HC)
        # query-block columns (0..512) of every chunk in one strided DMA:
        # unblocks the q projection and the first k block early
        nc.sync.dma_start(out=xT_v[:, :, 0:512], in_=xT_d[:, :, 0:512])
        wq0 = w_pool.tile([P, HC * P], F8, tag="wq_m")
        nc.sync.dma_start(out=wq0, in_=wq_d[0])
        wk0 = w_pool.tile([P, HC * P], F8, tag="wk_m")
        nc.sync.dma_start(out=wk0, in_=wk_d[0])
        bqk = consts.tile([P, 16], F32, tag="bqk")
        nc.sync.dma_start(out=bqk, in_=bqk_d[:, :])
        mk = consts.tile([P, NKT], F32, tag="mk")
        nc.sync.dma_start(out=mk, in_=mk_d[:, :])
        mks = consts.tile([P, NKT], F32, tag="mks")
        nc.sync.dma_start(out=mks, in_=mks_d[:, :])

        # ---- small consts / VA init (Pool) ----
        eps_t = consts.tile([P, 1], F32, tag="eps")
        nc.vector.memset(eps_t, EPS)
        ones32 = consts.tile([1, 64], F32, tag="ones32")
        nc.vector.memset(ones32, WS)  # 32.0: folds the fp8 ctxT scale
        ones32_r = ones32.bitcast(F32R)

        VA = va_pool.tile([P, VA_TOTAL], F8, name="VA", tag="VA")
        for t2 in range(8):
            for j in range(2):
                base = VA_T2 * t2 + VA_J * j
                # zero the pad block (cols 64..95)
                nc.gpsimd.memset(
                    _ap(VA, base + 64, [[VA_M, 8], [VA_H, 2], [1, 32]]), 0.0
                )
                # denominator col (64) = 1.0 per (m, h)
                nc.gpsimd.memset(
                    _ap(VA, base + 64, [[VA_M, 8], [VA_H, 2], [1, 1]]), 1.0
                )

        ctxT_all = ctxT_pool.tile([P, NPAIR * SQ], F8, name="ctxT", tag="ctxT")

        # ---- AP helpers ----
        def xt_rhs(c2, s0, ns):
            return _ap(xT_all, 2 * c2 * S + s0, [[S, 2], [1, ns]])

        def xt_lhsT(c2, t):
            return _ap(xT_all, 2 * c2 * S + t * P, [[S, 2], [1, P]])

        def w_lhsT(w, c2):
            return _ap(w, 2 * c2 * P, [[P, 2], [1, P]])

        def wv_rhs(g, c2):
            return _ap(wv_g[g], 2 * c2 * 512, [[512, 2], [1, 512]])

        def va_lhsT(t2, m, h):
            return _ap(VA, VA_T2 * t2 + VA_M * m + VA_H * h, [[VA_J, 2], [1, VA_C]])

        def va_dst(t, g):
            return _ap(
                VA,
                VA_T2 * (t // 2) + VA_J * (t % 2) + VA_M * (4 * g),
                [[VA_M, 4], [VA_H, 2], [1, 64]],
            )

        def qkdr_k_lhsT(qkdr, h, t):
            return _ap(qkdr, QKDR_H * h + 512 + t * P, [[QKDR_J, 2], [1, P]])

        def qkdr_q_rhs(qkdr, h):
            return _ap(qkdr, QKDR_H * h, [[QKDR_J, 2], [1, 512]])

        def pt_rhs(pt, h):
            return _ap(pt, 512 * h, [[1024, 2], [1, 512]])

        def ctxT_lhsT(c2, st):
            return _ap(ctxT_all, 2 * c2 * SQ + st * P, [[SQ, 2], [1, P]])

        # ---- elementwise engine dispatch ----
        def ew_evac_bias(eng, out, in0, bias_ap):
            # out = in0 + bias (per-partition), fp8 cast
            if eng == "a":
                nc.scalar.activation(
                    out, in0, mybir.ActivationFunctionType.Identity, bias=bias_ap
                )
            else:
                nc.vector.tensor_scalar(
                    out=out, in0=in0, scalar1=bias_ap, scalar2=None, op0=ADD
                )

        def ew_evac_scale(eng, out, in0, scale):
            if eng == "a":
                nc.scalar.mul(out, in0, scale)
            else:
                nc.vector.tensor_scalar(
                    out=out, in0=in0, scalar1=scale, scalar2=None, op0=MULT
                )

        def emit_exp(m, t, dst, s2):
            if t in EXP_A[m]:
                nc.scalar.activation(
                    dst,
                    s2,
                    mybir.ActivationFunctionType.Exp,
                    scale=EXP_SCALE,
                    bias=mk[:, t : t + 1],
                )
            else:
                # Schraudolph exp2 on DVE: u8 = s*A + (56 + 8*log2e*mask)
                # bitcast fp8e4 (piecewise-linear exp approx)
                nc.vector.tensor_scalar(
                    out=dst.bitcast(U8),
                    in0=s2,
                    scalar1=SCH_A,
                    scalar2=mks[:, t : t + 1],
                    op0=MULT,
                    op1=ADD,
                )

        # ---- per-pair q/k projection into one qk_sb + 4-DMA rearrange ----
        def emit_w_dma(m):
            wq_m = w_pool.tile([P, HC * P], F8, tag="wq_m")
            nc.sync.dma_start(out=wq_m, in_=wq_d[m])
            wk_m = w_pool.tile([P, HC * P], F8, tag="wk_m")
            nc.sync.dma_start(out=wk_m, in_=wk_d[m])
            return wq_m, wk_m

        def emit_q(m, wq_m, qk_sb, eng):
            """Emit q-proj matmuls now; return the evac as a closure so the
            caller can place it where the engine queue will find it ready."""
            ps = ps_mm.tile([P, 512], F32, name="ps", tag="ps")
            for c2 in range(4):
                nc.tensor.matmul(
                    ps,
                    w_lhsT(wq_m, c2),
                    xt_rhs(c2, 0, 512),
                    start=(c2 == 0),
                    stop=(c2 == 3),
                    perf_mode=DR,
                )
            return lambda: ew_evac_bias(
                eng, qk_sb[:, 0:512], ps, bqk[:, m : m + 1]
            )

        def emit_k_block(m, wk_m, qk_sb, n, eng):
            ps = ps_mm.tile([P, 512], F32, name="ps", tag="ps")
            for c2 in range(4):
                nc.tensor.matmul(
                    ps,
                    w_lhsT(wk_m, c2),
                    xt_rhs(c2, n * 512, 512),
                    start=(c2 == 0),
                    stop=(c2 == 3),
                    perf_mode=DR,
                )
            return lambda: ew_evac_bias(
                eng,
                qk_sb[:, 512 + n * 512 : 512 + (n + 1) * 512],
                ps,
                bqk[:, 8 + m : 9 + m],
            )

        def emit_qkdr_dma(qk_sb, qkdr):
            for h in range(2):
                for j in range(2):
                    nc.sync.dma_start(
                        out=qkdr[:, QKDR_H * h + QKDR_J * j :
                                 QKDR_H * h + QKDR_J * j + QKW],
                        in_=qk_sb[64 * h + 32 * j : 64 * h + 32 * j + 32, :],
                    )

        def emit_v(g, t):
            ps = ps_mm.tile([P, 512], F32, name="vps", tag="ps")
            for c2 in range(4):
                nc.tensor.matmul(
                    ps,
                    xt_lhsT(c2, t),
                    wv_rhs(g, c2),
                    start=(c2 == 0),
                    stop=(c2 == 3),
                    perf_mode=DR,
                )
            # wv is pre-scaled x32 on the host, so this is a pure fp8 cast
            return lambda: ew_evac_scale(
                V_ENG[g][t],
                va_dst(t, g),
                _ap(ps, 0, [[P, 4], [64, 2], [1, 64]]),
                1.0,
            )

        # ---- ctx normalization: ctxT = ctx_ps * (32/den), staged across
        # slots so the DVE/Act queue never sees a multi-op lump ----
        def norm_stage(st, stage):
            m, ctx_ps, store = st
            if stage == 0:
                store["rrs"] = []
                for h in range(2):
                    rr = r_pool.tile([1, 512], F32R, tag="rr")
                    nc.vector.reciprocal(rr, ctx_ps[h][64:65, :])
                    store["rrs"].append(rr)
            elif stage == 1:
                store["rbs"] = []
                for h in range(2):
                    bc_ps = ps_mm.tile([64, 512], F32, name="bc_ps", tag="ps")
                    nc.tensor.matmul(
                        bc_ps, ones32_r, store["rrs"][h], start=True, stop=True
                    )
                    # PSUM->SBUF broadcast copy on Act (one PSUM input per op)
                    rb = r_pool.tile([64, 512], F32, tag="rb")
                    nc.scalar.mul(rb, bc_ps, 1.0)
                    store["rbs"].append(rb)
            else:
                h = stage - 2
                nc.vector.tensor_tensor(
                    out=ctxT_all[64 * h : 64 * h + 64, m * SQ : (m + 1) * SQ],
                    in0=ctx_ps[h][0:64, :],
                    in1=store["rbs"][h],
                    op=MULT,
                )

        def emit_norm(m, ctx_ps):
            st = (m, ctx_ps, {})
            for stage in range(4):
                norm_stage(st, stage)

        # ---- main loop ----
        wv_g = []
        qk_sb0 = kv_pool.tile([P, QKW], F8, tag="qk_sb", name="qk_sb0")
        emit_q(0, wq0, qk_sb0, "d")()
        emit_k_block(0, wk0, qk_sb0, 0, "d")()
        # wv group 0 must beat pair 0's first V tiles; k-block columns of xT
        # next (pair 0 scores run straight off them), then the rest
        wvt0 = wv_pool.tile([P, HC * 512], F8, name="wv0", tag="wv0")
        nc.sync.dma_start(out=wvt0, in_=wv_d[0])
        wv_g.append(wvt0)
        nc.sync.dma_start(out=xT_v[:, :, 512:1024], in_=xT_d[:, :, 512:1024])
        # pair 1's weights early so its q/k proj (during pair 0) never waits
        w_next = emit_w_dma(1)
        nc.sync.dma_start(out=xT_v[:, :, 1024:1536], in_=xT_d[:, :, 1024:1536])
        nc.sync.dma_start(out=xT_v[:, :, 1536:S], in_=xT_d[:, :, 1536:S])
        wvt1 = wv_pool.tile([P, HC * 512], F8, name="wv1", tag="wv1")
        nc.sync.dma_start(out=wvt1, in_=wv_d[1])
        wv_g.append(wvt1)

        qk_cur, dr_cur = qk_sb0, None
        prev_norm = None  # staged normalization state of the previous pair
        qk_next = None
        dr_next = None
        ev_pend = []  # evac closures whose mm ran in the previous slot
        for m in range(NPAIR):
            if DEBUG and m == 2:
                nc.sync.dma_start(out=dqk_d[:, :], in_=qk_cur)
                nc.sync.dma_start(out=dqkdr_d[:, :], in_=dr_cur)
            ctx_ps = [
                ps_ctx.tile([P, 512], F32, name=f"ctx{h}", tag="ctx_ps")
                for h in range(2)
            ]

            def emit_ctx(e_t2, e_pt):
                for h in range(2):
                    nc.tensor.matmul(
                        ctx_ps[h][0:96, :],
                        va_lhsT(e_t2, m, h),
                        pt_rhs(e_pt, h),
                        start=(e_t2 == 0),
                        stop=(e_t2 == 7),
                        perf_mode=DR,
                    )

            pend_ctx = []
            for t2 in range(8):
                ev_now, ev_pend = ev_pend, []
                # --- phase 1: independent mms (V / next-pair proj / pair-0 k)
                # at the PE queue front: they are ready, while the slot's
                # score mms wait on the exp chain (strict in-order queues)
                evs = []
                if m == 0:
                    evs.append(emit_v(0, 2 * t2))
                    evs.append(emit_v(0, 2 * t2 + 1))
                elif m in V_SCHED and t2 in V_SCHED[m]:
                    for tv in V_SCHED[m][t2]:
                        evs.append(emit_v(1, tv))
                if m < NPAIR - 1:
                    # next pair's q/k proj: q at slot 0 (weights prefetched a
                    # whole pair ago), k blocks at slots 1-4, merged qkdr DMA
                    # at slot 5 right after k3's evac
                    if t2 == 0:
                        qk_next = kv_pool.tile(
                            [P, QKW], F8, tag="qk_sb", name="qk_sb"
                        )
                        ev_pend.append(
                            emit_q(m + 1, w_next[0], qk_next, Q_ENG[m])
                        )
                    elif t2 in (1, 2, 3, 4):
                        ev_pend.append(
                            emit_k_block(
                                m + 1, w_next[1], qk_next, t2 - 1,
                                K_ENG[m][t2 - 1],
                            )
                        )
                # norm broadcast matmuls ride the mm phase of slot 1
                if t2 == 1 and prev_norm is not None:
                    norm_stage(prev_norm, 1)
                # --- phase 2: scores + exps (the s2 double-buffer chain),
                # with ready evacs interleaved after the exps
                pt = pt_pool.tile([P, 2048], F8, name="pt", tag="pt")
                for j in range(2):
                    t = 2 * t2 + j
                    s2 = ps_s.tile([P, 1024], F32, name="s2", tag="s2")
                    for h in range(2):
                        if t < NODR[m]:
                            nc.tensor.matmul(
                                s2[:, 512 * h : 512 * (h + 1)],
                                qk_cur[64 * h : 64 * h + 64,
                                       512 + t * P : 512 + (t + 1) * P],
                                qk_cur[64 * h : 64 * h + 64, 0:512],
                                start=True,
                                stop=True,
                            )
                        else:
                            nc.tensor.matmul(
                                s2[:, 512 * h : 512 * (h + 1)],
                                qkdr_k_lhsT(dr_cur, h, t),
                                qkdr_q_rhs(dr_cur, h),
                                start=True,
                                stop=True,
                                perf_mode=DR,
                            )
                    emit_exp(m, t, pt[:, 1024 * j : 1024 * (j + 1)], s2)
                    if j == 0:
                        for ev in ev_now:
                            ev()
                if DEBUG and m == 3 and t2 == 7:
                    nc.sync.dma_start(out=dpt_d[:, :], in_=pt)
                # --- phase 3: this slot's evacs (mms ran in phase 1)
                for ev in evs:
                    ev()
                # --- phase 4: staged prev-pair normalization on DVE/Act
                if prev_norm is not None:
                    if t2 == 0:
                        norm_stage(prev_norm, 0)
                    elif t2 == 2:
                        norm_stage(prev_norm, 2)
                    elif t2 == 3:
                        norm_stage(prev_norm, 3)
                        prev_norm = None
                # --- phase 5: DMAs (SP queue)
                if m < NPAIR - 1:
                    if t2 == 0 and m < NPAIR - 2:
                        w_next2 = emit_w_dma(m + 2)
                    elif t2 == 5:
                        dr_next = kv_pool.tile(
                            [32, 2 * QKDR_H], F8, tag="qkdr", name="qkdr"
                        )
                        if m != 0:
                            emit_qkdr_dma(qk_next, dr_next)
                if m == 0 and t2 == 4:
                    # pair 0's own rearrange: its k blocks are all evacuated
                    # by slot 3, so scores t >= NODR[0] can run DoubleRow
                    dr_cur = kv_pool.tile(
                        [32, 2 * QKDR_H], F8, tag="qkdr", name="qkdr0"
                    )
                    emit_qkdr_dma(qk_sb0, dr_cur)
                if m == 0 and t2 == 7:
                    # pair 1 runs its first half non-DR; its qkdr DMA issues
                    # here (full qk_sb just evacuated) and lands by its t=8
                    emit_qkdr_dma(qk_next, dr_next)
                if t2 == 6 and 2 <= m <= 6:
                    # deferred non-critical input DMAs, spread one pair apart
                    if m == 2:
                        xq = []
                    if m <= 5:
                        xqt = xq_pool.tile(
                            [P, H], F32, name=f"xq{m-2}", tag=f"xq{m-2}"
                        )
                        nc.sync.dma_start(
                            out=xqt, in_=xq_d[(m - 2) * P : (m - 1) * P, :]
                        )
                        xq.append(xqt)
                    if m == 3 and not ln_trivial:
                        gamma_bc = consts.tile([P, H], F32, tag="gamma_bc")
                        nc.sync.dma_start(
                            out=gamma_bc, in_=gamma_d[:].partition_broadcast(P)
                        )
                    elif m == 4 and not ln_trivial:
                        beta_bc = consts.tile([P, H], F32, tag="beta_bc")
                        nc.sync.dma_start(
                            out=beta_bc, in_=beta_d[:].partition_broadcast(P)
                        )
                    elif m == 6:
                        wo_sb = wv_pool.tile([P, HC * H], F8, name="wo", tag="wo")
                        nc.sync.dma_start(out=wo_sb, in_=wo_d[:, :])
                # --- phase 6: PE queue tail: pair 0's own late k blocks
                # (their xT columns are still in flight at slot start) and
                # deferred ctx accumulation
                if m == 0 and t2 in (0, 1, 2):
                    ev_pend.append(
                        emit_k_block(0, wk0, qk_sb0, t2 + 1, K0_ENG[t2])
                    )
                pend_ctx.append((t2, pt))
                if len(pend_ctx) > CTX_DEFER:
                    emit_ctx(*pend_ctx.pop(0))
            for ev in ev_pend:
                ev()
            ev_pend = []
            for e in pend_ctx:
                emit_ctx(*e)
            prev_norm = (m, ctx_ps, {})
            qk_cur, dr_cur = qk_next, dr_next
            if m < NPAIR - 1:
                w_next = w_next2 if m < NPAIR - 2 else None
        for stage in range(4):
            norm_stage(prev_norm, stage)

        def wo_rhs(c2, nch):
            return _ap(wo_sb, 2 * c2 * H + nch * 512, [[H, 2], [1, 512]])

        if DEBUG:
            nc.sync.dma_start(out=dva_d[:, :], in_=VA)
            nc.sync.dma_start(out=dctxT_d[:, :], in_=ctxT_all)
        # ---- output projection + residual + LayerNorm ----
        # LN stats come from accum_out side-channels: the residual stts
        # accumulate sum(h) per half for free; Act Square passes give
        # sum(h^2); tiny [P,1] ops combine to mean/var (replaces bn_stats).
        inv_h = 1.0 / H
        for st in range(SQ // P):
            ps = ps_s.tile([P, 1024], F32, name="ops", tag="s2")
            for nch in range(2):
                for c2 in range(4):
                    nc.tensor.matmul(
                        ps[:, nch * 512 : (nch + 1) * 512],
                        ctxT_lhsT(c2, st),
                        wo_rhs(c2, nch),
                        start=(c2 == 0),
                        stop=(c2 == 3),
                        perf_mode=DR,
                    )
            h_sb = ln_pool.tile([P, H], F32, tag="h_sb")
            acc = ln_pool.tile([P, 2], F32, tag="acc")
            for nch in range(2):
                # fused evac: h = ps * 2^-15 + (x + bo + bv@Wo); sum(h) is a
                # free accum side-channel on DVE
                nc.vector.scalar_tensor_tensor(
                    out=h_sb[:, nch * 512 : (nch + 1) * 512],
                    in0=ps[:, nch * 512 : (nch + 1) * 512],
                    scalar=OUT_SCALE,
                    in1=xq[st][:, nch * 512 : (nch + 1) * 512],
                    op0=MULT,
                    op1=ADD,
                    accum_out=acc[:, nch : nch + 1],
                )
            u = ln_pool.tile([P, 1], F32, tag="u")
            nc.vector.tensor_scalar(
                out=u, in0=acc[:, 0:1], scalar1=acc[:, 1:2], scalar2=inv_h,
                op0=ADD, op1=MULT,
            )
            if st in VAR_SQ_STS:
                var = ln_pool.tile([P, 1], F32, tag="var")
                # E[h^2] via Act Square passes (accum side-channel)
                ssq = ln_pool.tile([P, 2], F32, tag="ssq")
                sqj = ln_pool.tile([P, H], F32, tag="sqj")
                for nch in range(2):
                    nc.scalar.activation(
                        sqj[:, nch * 512 : (nch + 1) * 512],
                        h_sb[:, nch * 512 : (nch + 1) * 512],
                        mybir.ActivationFunctionType.Square,
                        accum_out=ssq[:, nch : nch + 1],
                    )
                m2 = ln_pool.tile([P, 1], F32, tag="m2")
                nc.vector.tensor_scalar(
                    out=m2, in0=ssq[:, 0:1], scalar1=ssq[:, 1:2],
                    scalar2=inv_h, op0=ADD, op1=MULT,
                )
                u2 = ln_pool.tile([P, 1], F32, tag="u2")
                nc.vector.tensor_tensor(out=u2, in0=u, in1=u, op=MULT)
                nc.vector.tensor_tensor(out=var, in0=m2, in1=u2, op=SUB)
            else:
                stats = ln_pool.tile([P, 2, 6], F32, tag="stats")
                for gg in range(2):
                    nc.vector.bn_stats(
                        out=stats[:, gg, :],
                        in_=h_sb[:, gg * 512 : (gg + 1) * 512],
                    )
                mv = ln_pool.tile([P, 2], F32, tag="mv")
                nc.vector.bn_aggr(out=mv, in_=stats)
                var = mv[:, 1:2]
            sd = ln_pool.tile([P, 1], F32, tag="sd")
            nc.scalar.activation(
                sd, var, mybir.ActivationFunctionType.Sqrt, bias=eps_t
            )
            rs = ln_pool.tile([P, 1], F32, tag="rs")
            nc.vector.reciprocal(rs, sd)
            if DEBUG and st == 0:
                nc.sync.dma_start(out=dh_d[:, :], in_=h_sb)
            if ln_trivial:
                # gamma == 1, beta == 0: out = (h - u) * rsig directly, in
                # halves (DVE + Pool) so the first output DMA overlaps the
                # second half's compute
                ob = ln_pool.tile([P, H], F32, tag="ob")
                nc.vector.tensor_scalar(
                    out=ob[:, 0:512], in0=h_sb[:, 0:512],
                    scalar1=u, scalar2=rs, op0=SUB, op1=MULT,
                )
                nc.sync.dma_start(
                    out=out_d[st * P : (st + 1) * P, 0:512], in_=ob[:, 0:512]
                )
                nc.gpsimd.tensor_scalar(
                    out=ob[:, 512:1024], in0=h_sb[:, 512:1024],
                    scalar1=u, scalar2=rs, op0=SUB, op1=MULT,
                )
                nc.sync.dma_start(
                    out=out_d[st * P : (st + 1) * P, 512:1024],
                    in_=ob[:, 512:1024],
                )
            else:
                # t1 = (h - u) * gamma ; ob = t1 * rsig + beta, halves
                # split across DVE and Pool
                t1 = ln_pool.tile([P, H], F32, tag="t1")
                ob = ln_pool.tile([P, H], F32, tag="ob")
                nc.vector.scalar_tensor_tensor(
                    out=t1[:, 0:512], in0=h_sb[:, 0:512], scalar=u,
                    in1=gamma_bc[:, 0:512], op0=SUB, op1=MULT,
                )
                nc.gpsimd.scalar_tensor_tensor(
                    out=ob[:, 0:512], in0=t1[:, 0:512], scalar=rs,
                    in1=beta_bc[:, 0:512], op0=MULT, op1=ADD,
                )
                nc.sync.dma_start(
                    out=out_d[st * P : (st + 1) * P, 0:512], in_=ob[:, 0:512]
                )
                nc.gpsimd.scalar_tensor_tensor(
                    out=t1[:, 512:1024], in0=h_sb[:, 512:1024], scalar=u,
                    in1=gamma_bc[:, 512:1024], op0=SUB, op1=MULT,
                )
                nc.vector.scalar_tensor_tensor(
                    out=ob[:, 512:1024], in0=t1[:, 512:1024], scalar=rs,
                    in1=beta_bc[:, 512:1024], op0=MULT, op1=ADD,
                )
                nc.sync.dma_start(
                    out=out_d[st * P : (st + 1) * P, 512:1024],
                    in_=ob[:, 512:1024],
                )

        for _pool in (ps_ctx, ps_s, ps_mm, ln_pool, r_pool, pt_pool, kv_pool,
                      w_pool, xq_pool, ctxT_pool, wv_pool, va_pool, xT_pool,
                      consts):
            _pool.release()

    _split_excess_waits(nc)
    return nc


_NC = {}


def _get_nc(ln_trivial=True):
    if ln_trivial not in _NC:
        _NC[ln_trivial] = build_nc(ln_trivial)
    return _NC[ln_trivial]


def _in_maps(hidden_states, attention_mask, Wq, bq, Wk, bk, Wv, bv, Wo, bo, gamma, beta):
    f8 = ml_dtypes.float8_e4m3
    hs = np.asarray(hidden_states, dtype=np.float32)
    am = np.asarray(attention_mask, dtype=np.float32).reshape(2, S)
    Wo_f = np.asarray(Wo, dtype=np.float32)

    def pair_w(w):
        w = np.asarray(w, dtype=np.float32) * WS
        return np.ascontiguousarray(
            w.reshape(HC, P, NPAIR, P).transpose(2, 1, 0, 3).reshape(NPAIR, P, H)
        ).astype(f8)

    wq_t, wk_t = pair_w(Wq), pair_w(Wk)
    wv_t = np.ascontiguousarray(
        (np.asarray(Wv, dtype=np.float32) * WS)
        .reshape(HC, P, 2, 512)
        .transpose(2, 1, 0, 3)
        .reshape(2, P, HC * 512)
    ).astype(f8)
    wo_t = np.ascontiguousarray(
        (Wo_f * WS).reshape(HC, P, H).transpose(1, 0, 2).reshape(P, HC * H)
    ).astype(f8)
    bqk = np.ascontiguousarray(
        np.concatenate(
            [
                (np.asarray(b, dtype=np.float32) * WS).reshape(NPAIR, P).T
                for b in (bq, bk)
            ],
            axis=1,
        )
    )
    g_c = np.ascontiguousarray(np.asarray(gamma, dtype=np.float32))
    be_c = np.ascontiguousarray(np.asarray(beta, dtype=np.float32))
    # residual folds: x + bo + bv @ Wo
    res_c = (
        np.asarray(bo, dtype=np.float32)
        + np.asarray(bv, dtype=np.float32) @ Wo_f
    )

    maps = []
    for core in range(8):
        b, j = core // 4, core % 4
        # roll the sequence so this core's query block is always cols [0, 512);
        # attention sums over all keys, so key order is irrelevant as long as
        # the additive mask is rolled identically.
        xs = np.roll(hs[b], -j * SQ, axis=0)
        ms = np.roll(am[b], -j * SQ)
        xT = np.ascontiguousarray(
            xs.T.reshape(HC, P, S).transpose(1, 0, 2)
        ).astype(f8)
        mk_c = np.ascontiguousarray(ms.reshape(NKT, P).T)
        maps.append(
            {
                "xT": xT,
                "xq": np.ascontiguousarray(xs[0:SQ, :] + res_c),
                "wq": wq_t,
                "wk": wk_t,
                "wv": wv_t,
                "wo": wo_t,
                "bqk": bqk,
                "gamma": g_c,
                "beta": be_c,
                "mk": mk_c,
                "mks": np.ascontiguousarray(SCH_B + 8 * 1.4426950408889634 * mk_c),
            }
        )
    return maps


def run(trace=False, **inputs):
    ln_trivial = bool(
        np.all(np.asarray(inputs["gamma"]) == 1.0)
        and np.all(np.asarray(inputs["beta"]) == 0.0)
    )
    nc = _get_nc(ln_trivial)
    maps = _in_maps(**inputs)
    res = run_bass_kernel_spmd(nc, maps, core_ids=list(range(8)), trace=trace)
    out = np.empty((2, S, H), dtype=np.float32)
    for core in range(8):
        b, j = core // 4, core % 4
        out[b, j * SQ : (j + 1) * SQ, :] = res.results[core]["out"]
    return out, res


def kernel(**inputs):
    out, _ = run(trace=False, **inputs)
    return out
